# revision 4
# baseline (speedup 1.0000x reference)
"""Trainium2 Bass kernel for nn_AdversarialAttack (brute-force 1-NN over vocab).

Strategy (8 NeuronCores, vocab-sharded):
  - Each core holds a [4000, 2048] shard of the embedding table (padded to
    4096 rows), transposed+blocked for the PE, in bf16 (plus a bf16 residual
    "lo" part for high-precision rescoring).
  - Main sweep: for all 8192 query embeddings, compute
        m[q, v] = <x_q, w_v> - |w_v|^2 / 2
    via bf16 matmuls accumulated in fp32 PSUM, add the -|w|^2/2 bias with the
    DVE while moving PSUM->SBUF, then take per-query argmax over the local
    vocab with the DVE max8/max_index instructions.  argmax_v m  ==  argmin_v
    ||x - w_v||^2.  For non-attack positions the top-1 margin is ~2000 sigma,
    so bf16 scoring is exact.
  - Attack positions hold one of only 20 distinct attack vectors.  Those are
    rescored with a hi/lo split-precision pass (x_hi*w_hi + x_lo*w_hi +
    x_hi*w_lo, all bf16 matmuls, fp32 accumulate) giving ~fp32 accuracy.
  - Host side: embedding gather / attack scatter (pure data movement), the
    final all-reduce argmin over the 8 local argmins, and the attack-position
    override.
"""

import numpy as np
import ml_dtypes

# ---- problem constants (hardcoded per task instructions) ----
B, S, V, D, N_ATK = 4, 2048, 32000, 2048, 20
N_CORES = 8
P = 128
KT = D // P               # 16 contraction tiles
BSQ = B * S               # 8192 queries
QT = BSQ // P             # 64 query tiles
VSH = V // N_CORES        # 4000 vocab rows per core
VPAD = 4096               # padded vocab rows per core
NV = VPAD // 512          # 8 v-tiles of 512
NA = 32                   # attack rows padded (20 real)
NEG_BIG = -1.0e30

_CACHE = {}


def _build_nc(reps: int = 1):
    import concourse.bacc as bacc
    import concourse.mybir as mybir
    from concourse.tile import TileContext

    dt = mybir.dt
    nc = bacc.Bacc("TRN2")

    # per-core inputs (already laid out for SBUF: leading dim = partition)
    xt_d = nc.dram_tensor("xt", [QT, P, KT, P], dt.bfloat16, kind="ExternalInput")
    wt_d = nc.dram_tensor("wt", [P, NV, KT, 512], dt.bfloat16, kind="ExternalInput")
    wlo_d = nc.dram_tensor("wlo", [NV, KT, P, 512], dt.bfloat16, kind="ExternalInput")
    xat_d = nc.dram_tensor("xat", [P, KT, NA], dt.bfloat16, kind="ExternalInput")
    xalo_d = nc.dram_tensor("xalo", [P, KT, NA], dt.bfloat16, kind="ExternalInput")
    w2_d = nc.dram_tensor("w2rep", [P, VPAD], dt.float32, kind="ExternalInput")

    mval_d = nc.dram_tensor("mval", [QT, P], dt.float32, kind="ExternalOutput")
    midx_d = nc.dram_tensor("midx", [QT, P], dt.uint32, kind="ExternalOutput")
    aval_d = nc.dram_tensor("aval", [NA], dt.float32, kind="ExternalOutput")
    aidx_d = nc.dram_tensor("aidx", [NA], dt.uint32, kind="ExternalOutput")

    with TileContext(nc) as tc:
        with (
            tc.tile_pool(name="wres", bufs=1) as wres_pool,
            tc.tile_pool(name="w2", bufs=1) as w2_pool,
            tc.tile_pool(name="att", bufs=1) as att_pool,
            tc.tile_pool(name="xq", bufs=3) as xq_pool,
            tc.tile_pool(name="mblk", bufs=2) as m_pool,
            tc.tile_pool(name="wlo", bufs=4) as wlo_pool,
            tc.tile_pool(name="small", bufs=4) as sm_pool,
            tc.tile_pool(name="psum", bufs=2, space="PSUM") as ps_pool,
        ):
            # resident tensors, loaded once
            w_res = wres_pool.tile([P, NV, KT, 512], dt.bfloat16)
            nc.sync.dma_start(w_res[:], wt_d[:])
            w2rep = w2_pool.tile([P, VPAD], dt.float32)
            nc.sync.dma_start(w2rep[:], w2_d[:])
            xat = att_pool.tile([P, KT, NA], dt.bfloat16)
            nc.sync.dma_start(xat[:], xat_d[:])
            xalo = att_pool.tile([P, KT, NA], dt.bfloat16)
            nc.sync.dma_start(xalo[:], xalo_d[:])

            for _rep in range(reps):
                # ---------------- main sweep ----------------
                for qt in range(QT):
                    xq = xq_pool.tile([P, KT, P], dt.bfloat16, tag="xq")
                    nc.sync.dma_start(xq[:], xt_d[qt])
                    mblk = m_pool.tile([P, VPAD], dt.bfloat16, tag="mblk")
                    for half in range(2):
                        psums = [
                            ps_pool.tile([P, 512], dt.float32, tag=f"ps{i}", name=f"ps{i}")
                            for i in range(4)
                        ]
                        for k in range(KT):
                            for i in range(4):
                                vt = half * 4 + i
                                nc.tensor.matmul(
                                    psums[i][:],
                                    xq[:, k, :],
                                    w_res[:, vt, k, :],
                                    start=(k == 0),
                                    stop=(k == KT - 1),
                                )
                        for i in range(4):
                            vt = half * 4 + i
                            sl = slice(vt * 512, (vt + 1) * 512)
                            nc.vector.tensor_add(
                                out=mblk[:, sl], in0=psums[i][:], in1=w2rep[:, sl]
                            )
                    mx = sm_pool.tile([P, 8], dt.bfloat16, tag="mx")
                    mi = sm_pool.tile([P, 8], dt.uint32, tag="mi")
                    nc.vector.max(out=mx[:], in_=mblk[:])
                    nc.vector.max_index(out=mi[:], in_max=mx[:], in_values=mblk[:])
                    mvf = sm_pool.tile([P, 1], dt.float32, tag="mvf")
                    nc.scalar.copy(mvf[:], mx[:, 0:1])
                    nc.sync.dma_start(mval_d[qt, :, None], mvf[:])
                    nc.sync.dma_start(midx_d[qt, :, None], mi[:, 0:1])

                # ---------------- attack rescore (split precision) ----------------
                matt = att_pool.tile([NA, VPAD], dt.float32, tag="matt")
                for vt in range(NV):
                    # share PSUM slots with the main sweep's ps0 tag
                    ps = ps_pool.tile([NA, 512], dt.float32, tag="ps0", name="aps")
                    for k in range(KT):
                        wlo_t = wlo_pool.tile([P, 512], dt.bfloat16, tag="wlo")
                        nc.sync.dma_start(wlo_t[:], wlo_d[vt, k])
                        nc.tensor.matmul(
                            ps[:], xat[:, k, :], w_res[:, vt, k, :],
                            start=(k == 0), stop=False,
                        )
                        nc.tensor.matmul(
                            ps[:], xalo[:, k, :], w_res[:, vt, k, :],
                            start=False, stop=False,
                        )
                        nc.tensor.matmul(
                            ps[:], xat[:, k, :], wlo_t[:],
                            start=False, stop=(k == KT - 1),
                        )
                    sl = slice(vt * 512, (vt + 1) * 512)
                    nc.vector.tensor_add(
                        out=matt[:, sl], in0=ps[:], in1=w2rep[:NA, sl]
                    )
                amx = sm_pool.tile([NA, 8], dt.float32, tag="amx")
                ami = sm_pool.tile([NA, 8], dt.uint32, tag="ami")
                nc.vector.max(out=amx[:], in_=matt[:])
                nc.vector.max_index(out=ami[:], in_max=amx[:], in_values=matt[:])
                nc.sync.dma_start(aval_d[:, None], amx[:, 0:1])
                nc.sync.dma_start(aidx_d[:, None], ami[:, 0:1])

    nc.finalize()
    return nc


class _SpmdRunner:
    """Compile-once / run-many mirror of bass2jax.run_bass_via_pjrt."""

    def __init__(self, nc, n_cores):
        import jax
        import concourse.mybir as mybir
        from jax.sharding import Mesh, PartitionSpec
        from jax.experimental.shard_map import shard_map
        from concourse.bass2jax import (
            _bass_exec_p,
            install_neuronx_cc_hook,
            partition_id_tensor,
        )

        install_neuronx_cc_hook()
        self.jax = jax
        self.n_cores = n_cores
        partition_name = (
            nc.partition_id_tensor.name if nc.partition_id_tensor else None
        )
        in_names, out_names, out_avals, zero_outs = [], [], [], []
        for alloc in nc.m.functions[0].allocations:
            if not isinstance(alloc, mybir.MemoryLocationSet):
                continue
            name = alloc.memorylocations[0].name
            if alloc.kind == "ExternalInput":
                if name != partition_name:
                    in_names.append(name)
            elif alloc.kind == "ExternalOutput":
                out_names.append(name)
                shape = tuple(alloc.tensor_shape)
                dtype = mybir.dt.np(alloc.dtype)
                out_avals.append(jax.core.ShapedArray(shape, dtype))
                zero_outs.append(np.zeros(shape, dtype))
        self.in_names = in_names
        self.out_names = out_names
        self.out_avals = out_avals
        self.zero_outs = zero_outs
        n_params = len(in_names)
        n_outs = len(out_avals)
        all_in_names = in_names + out_names
        if partition_name is not None:
            all_in_names.append(partition_name)

        def _body(*args):
            operands = list(args)
            if partition_name is not None:
                operands.append(partition_id_tensor())
            outs = _bass_exec_p.bind(
                *operands,
                out_avals=tuple(out_avals),
                in_names=tuple(all_in_names),
                out_names=tuple(out_names),
                lowering_input_output_aliases=(),
                sim_require_finite=True,
                sim_require_nnan=True,
                nc=nc,
            )
            return tuple(outs)

        devices = jax.devices()[:n_cores]
        assert len(devices) == n_cores, f"need {n_cores} cores"
        self.mesh = Mesh(np.asarray(devices), ("core",))
        in_specs = (PartitionSpec("core"),) * (n_params + n_outs)
        out_specs = (PartitionSpec("core"),) * len(out_names)
        self.fn = jax.jit(
            shard_map(
                _body, mesh=self.mesh, in_specs=in_specs, out_specs=out_specs,
                check_rep=False,
            ),
            keep_unused=True,
        )

    def prepare(self, in_maps):
        import jax
        from jax.sharding import PartitionSpec

        n = self.n_cores
        concat = [
            np.concatenate([np.asarray(in_maps[c][name]) for c in range(n)], axis=0)
            for name in self.in_names
        ] + [
            np.zeros((n * z.shape[0], *z.shape[1:]), z.dtype) for z in self.zero_outs
        ]
        sharding = jax.sharding.NamedSharding(self.mesh, PartitionSpec("core"))
        return [jax.device_put(a, sharding) for a in concat]

    def run(self, args):
        out = self.fn(*args)
        self.jax.block_until_ready(out)
        return [
            {
                name: np.asarray(out[i]).reshape(
                    self.n_cores, *self.out_avals[i].shape
                )[c]
                for i, name in enumerate(self.out_names)
            }
            for c in range(self.n_cores)
        ]


def _get_runner(reps: int = 1):
    key = ("runner", reps)
    if key not in _CACHE:
        nc = _build_nc(reps)
        _CACHE[key] = _SpmdRunner(nc, N_CORES)
    return _CACHE[key]


def _prep_host(input_ids, attack_mask, attack, emb_weight):
    """Gather/scatter + per-core layout prep (host-side data movement)."""
    bf16 = ml_dtypes.bfloat16
    W = np.ascontiguousarray(np.asarray(emb_weight, dtype=np.float32))
    ids = np.asarray(input_ids).astype(np.int64)
    mask = np.asarray(attack_mask).astype(bool)
    A = np.asarray(attack, dtype=np.float32)

    # reference semantics: embed, then masked scatter of tiled attack rows
    X = W[ids.reshape(-1)].reshape(B, S, D).copy()
    rank = (np.cumsum(mask, axis=-1) - 1) % N_ATK
    X[mask] = A[rank[mask]]
    embeds = X.copy()  # fp32 exact output
    Xq = X.reshape(BSQ, D)

    # query side, bf16, blocked [QT, P(part=d), KT, P(free=q)]
    X_bf = Xq.astype(bf16)
    xt = np.ascontiguousarray(
        X_bf.reshape(QT, P, KT, P).transpose(0, 3, 2, 1)
    )  # [qt, p, k, j] = X[qt*128+j, k*128+p]

    # attack rows hi/lo, padded to 32, blocked [P, KT, NA]
    A_hi = A.astype(bf16)
    A_lo = (A - A_hi.astype(np.float32)).astype(bf16)

    def att_block(Ax):
        Ap = np.zeros((NA, D), dtype=bf16)
        Ap[:N_ATK] = Ax
        return np.ascontiguousarray(Ap.reshape(NA, KT, P).transpose(2, 1, 0))

    xat = att_block(A_hi)
    xalo = att_block(A_lo)

    in_maps = []
    for c in range(N_CORES):
        Wc = W[c * VSH : (c + 1) * VSH]
        Whi = Wc.astype(bf16)
        Wlo = (Wc - Whi.astype(np.float32)).astype(bf16)

        Whi_p = np.zeros((VPAD, D), dtype=bf16)
        Whi_p[:VSH] = Whi
        Wlo_p = np.zeros((VPAD, D), dtype=bf16)
        Wlo_p[:VSH] = Wlo
        # wt: [P(part=d), NV, KT, 512]; wlo: [NV, KT, P, 512]
        wt = np.ascontiguousarray(
            Whi_p.reshape(NV, 512, KT, P).transpose(3, 0, 2, 1)
        )
        wlo = np.ascontiguousarray(
            Wlo_p.reshape(NV, 512, KT, P).transpose(0, 2, 3, 1)
        )
        w2n = np.full((VPAD,), NEG_BIG, dtype=np.float32)
        w2n[:VSH] = -0.5 * np.einsum(
            "vd,vd->v", Wc.astype(np.float64), Wc.astype(np.float64)
        ).astype(np.float32)
        w2rep = np.ascontiguousarray(np.broadcast_to(w2n, (P, VPAD)))

        in_maps.append(
            {
                "xt": xt,
                "wt": wt,
                "wlo": wlo,
                "xat": xat,
                "xalo": xalo,
                "w2rep": w2rep,
            }
        )
    return embeds, rank, mask, in_maps


def _combine(results, rank, mask, ids_dtype):
    """Host-side all-reduce argmin over the 8 local argmins + attack override."""
    vals = np.stack([results[c]["mval"].reshape(BSQ) for c in range(N_CORES)])
    idxs = np.stack([results[c]["midx"].reshape(BSQ) for c in range(N_CORES)])
    best_core = np.argmax(vals, axis=0)  # ties -> lowest core == lowest vocab id
    adv = best_core * VSH + idxs[best_core, np.arange(BSQ)]

    avals = np.stack([results[c]["aval"][:N_ATK] for c in range(N_CORES)])
    aidxs = np.stack([results[c]["aidx"][:N_ATK] for c in range(N_CORES)])
    abest_core = np.argmax(avals, axis=0)
    att_ids = abest_core * VSH + aidxs[abest_core, np.arange(N_ATK)]

    adv = adv.reshape(B, S)
    adv[mask] = att_ids[rank[mask]]
    return adv.astype(ids_dtype)


def kernel(input_ids, attack_mask, attack, emb_weight):
    embeds, rank, mask, in_maps = _prep_host(
        input_ids, attack_mask, attack, emb_weight
    )
    runner = _get_runner(reps=1)
    args = runner.prepare(in_maps)
    results = runner.run(args)
    adv = _combine(results, rank, mask, np.asarray(input_ids).dtype)
    return embeds, adv


# revision 6
# speedup vs baseline: 15.4970x; 15.4970x over previous
"""Trainium2 Bass kernel for nn_AdversarialAttack (brute-force 1-NN over vocab).

Strategy (8 NeuronCores, vocab-sharded):
  - Each core holds a [4000, 2048] shard of the embedding table (padded to
    4096 rows), transposed+blocked for the PE, in bf16 (plus a bf16 residual
    "lo" part for high-precision rescoring).
  - Main sweep: for all 8192 query embeddings, compute
        m[q, v] = <x_q, w_v> - |w_v|^2 / 2
    via bf16 matmuls accumulated in fp32 PSUM, add the -|w|^2/2 bias with the
    DVE while moving PSUM->SBUF, then take per-query argmax over the local
    vocab with the DVE max8/max_index instructions.  argmax_v m  ==  argmin_v
    ||x - w_v||^2.  For non-attack positions the top-1 margin is ~2000 sigma,
    so bf16 scoring is exact.
  - Attack positions hold one of only 20 distinct attack vectors.  Those are
    rescored with a hi/lo split-precision pass (x_hi*w_hi + x_lo*w_hi +
    x_hi*w_lo, all bf16 matmuls, fp32 accumulate) giving ~fp32 accuracy.
  - Host side: embedding gather / attack scatter (pure data movement), the
    final all-reduce argmin over the 8 local argmins, and the attack-position
    override.
"""

import numpy as np
import ml_dtypes

# ---- problem constants (hardcoded per task instructions) ----
B, S, V, D, N_ATK = 4, 2048, 32000, 2048, 20
N_CORES = 8
P = 128
KT = D // P               # 16 contraction tiles
BSQ = B * S               # 8192 queries
QT = BSQ // P             # 64 query tiles
VSH = V // N_CORES        # 4000 vocab rows per core
VPAD = 4096               # padded vocab rows per core
NV = VPAD // 512          # 8 v-tiles of 512
NA = 32                   # attack rows padded (20 real)
NEG_BIG = -1.0e30

_CACHE = {}


def _build_nc(reps: int = 1, fp8: bool = True):
    import concourse.bacc as bacc
    import concourse.mybir as mybir
    from concourse.tile import TileContext

    dt = mybir.dt
    nc = bacc.Bacc("TRN2")
    sweep_dt = dt.float8e4 if fp8 else dt.bfloat16

    # per-core inputs (already laid out for SBUF: leading dim = partition)
    xt_d = nc.dram_tensor("xt", [QT, P, KT, P], sweep_dt, kind="ExternalInput")
    wt_d = nc.dram_tensor("wt", [P, NV, KT, 512], sweep_dt, kind="ExternalInput")
    whi_d = nc.dram_tensor("whi", [NV, KT, P, 512], dt.bfloat16, kind="ExternalInput")
    wlo_d = nc.dram_tensor("wlo", [NV, KT, P, 512], dt.bfloat16, kind="ExternalInput")
    xat_d = nc.dram_tensor("xat", [P, KT, NA], dt.bfloat16, kind="ExternalInput")
    xalo_d = nc.dram_tensor("xalo", [P, KT, NA], dt.bfloat16, kind="ExternalInput")
    w2_d = nc.dram_tensor("w2rep", [P, VPAD], dt.float32, kind="ExternalInput")

    mval_d = nc.dram_tensor("mval", [QT, P], dt.float32, kind="ExternalOutput")
    midx_d = nc.dram_tensor("midx", [QT, P], dt.uint32, kind="ExternalOutput")
    aval_d = nc.dram_tensor("aval", [NA], dt.float32, kind="ExternalOutput")
    aidx_d = nc.dram_tensor("aidx", [NA], dt.uint32, kind="ExternalOutput")

    with TileContext(nc) as tc:
        with (
            tc.tile_pool(name="wres", bufs=1) as wres_pool,
            tc.tile_pool(name="w2", bufs=1) as w2_pool,
            tc.tile_pool(name="att", bufs=1) as att_pool,
            tc.tile_pool(name="xq", bufs=3) as xq_pool,
            tc.tile_pool(name="mblk", bufs=2) as m_pool,
            tc.tile_pool(name="wlo", bufs=4) as wlo_pool,
            tc.tile_pool(name="small", bufs=4) as sm_pool,
            tc.tile_pool(name="psum", bufs=2, space="PSUM") as ps_pool,
        ):
            # resident tensors, loaded once
            w_res = wres_pool.tile([P, NV, KT, 512], sweep_dt)
            nc.sync.dma_start(w_res[:], wt_d[:])
            w2rep = w2_pool.tile([P, VPAD], dt.float32)
            nc.sync.dma_start(w2rep[:], w2_d[:])
            xat = att_pool.tile([P, KT, NA], dt.bfloat16)
            nc.sync.dma_start(xat[:], xat_d[:])
            xalo = att_pool.tile([P, KT, NA], dt.bfloat16)
            nc.sync.dma_start(xalo[:], xalo_d[:])

            for _rep in range(reps):
                # ---------------- main sweep ----------------
                for qt in range(QT):
                    xq = xq_pool.tile([P, KT, P], sweep_dt, tag="xq")
                    nc.sync.dma_start(xq[:], xt_d[qt])
                    mblk = m_pool.tile([P, VPAD], dt.bfloat16, tag="mblk")
                    for half in range(2):
                        psums = [
                            ps_pool.tile([P, 512], dt.float32, tag=f"ps{i}", name=f"ps{i}")
                            for i in range(4)
                        ]
                        if fp8:
                            for kp in range(KT // 2):
                                for i in range(4):
                                    vt = half * 4 + i
                                    nc.tensor.matmul(
                                        psums[i][:],
                                        xq[:, 2 * kp : 2 * kp + 2, :],
                                        w_res[:, vt, 2 * kp : 2 * kp + 2, :],
                                        start=(kp == 0),
                                        stop=(kp == KT // 2 - 1),
                                        perf_mode=mybir.MatmulPerfMode.DoubleRow,
                                    )
                        else:
                            for k in range(KT):
                                for i in range(4):
                                    vt = half * 4 + i
                                    nc.tensor.matmul(
                                        psums[i][:],
                                        xq[:, k, :],
                                        w_res[:, vt, k, :],
                                        start=(k == 0),
                                        stop=(k == KT - 1),
                                    )
                        for i in range(4):
                            vt = half * 4 + i
                            sl = slice(vt * 512, (vt + 1) * 512)
                            nc.vector.tensor_add(
                                out=mblk[:, sl], in0=psums[i][:], in1=w2rep[:, sl]
                            )
                    mx = sm_pool.tile([P, 8], dt.bfloat16, tag="mx")
                    mi = sm_pool.tile([P, 8], dt.uint32, tag="mi")
                    nc.vector.max(out=mx[:], in_=mblk[:])
                    nc.vector.max_index(out=mi[:], in_max=mx[:], in_values=mblk[:])
                    mvf = sm_pool.tile([P, 1], dt.float32, tag="mvf")
                    nc.scalar.copy(mvf[:], mx[:, 0:1])
                    nc.sync.dma_start(mval_d[qt, :, None], mvf[:])
                    nc.sync.dma_start(midx_d[qt, :, None], mi[:, 0:1])

                # ---------------- attack rescore (split precision) ----------------
                matt = att_pool.tile([NA, VPAD], dt.float32, tag="matt")
                for vt in range(NV):
                    # share PSUM slots with the main sweep's ps0 tag
                    ps = ps_pool.tile([NA, 512], dt.float32, tag="ps0", name="aps")
                    for k in range(KT):
                        whi_t = wlo_pool.tile([P, 512], dt.bfloat16, tag="whi")
                        nc.sync.dma_start(whi_t[:], whi_d[vt, k])
                        wlo_t = wlo_pool.tile([P, 512], dt.bfloat16, tag="wlo")
                        nc.sync.dma_start(wlo_t[:], wlo_d[vt, k])
                        nc.tensor.matmul(
                            ps[:], xat[:, k, :], whi_t[:],
                            start=(k == 0), stop=False,
                        )
                        nc.tensor.matmul(
                            ps[:], xalo[:, k, :], whi_t[:],
                            start=False, stop=False,
                        )
                        nc.tensor.matmul(
                            ps[:], xat[:, k, :], wlo_t[:],
                            start=False, stop=(k == KT - 1),
                        )
                    sl = slice(vt * 512, (vt + 1) * 512)
                    nc.vector.tensor_add(
                        out=matt[:, sl], in0=ps[:], in1=w2rep[:NA, sl]
                    )
                amx = sm_pool.tile([NA, 8], dt.float32, tag="amx")
                ami = sm_pool.tile([NA, 8], dt.uint32, tag="ami")
                nc.vector.max(out=amx[:], in_=matt[:])
                nc.vector.max_index(out=ami[:], in_max=amx[:], in_values=matt[:])
                nc.sync.dma_start(aval_d[:, None], amx[:, 0:1])
                nc.sync.dma_start(aidx_d[:, None], ami[:, 0:1])

    nc.finalize()
    return nc


class _SpmdRunner:
    """Compile-once / run-many mirror of bass2jax.run_bass_via_pjrt."""

    def __init__(self, nc, n_cores):
        import jax
        import concourse.mybir as mybir
        from jax.sharding import Mesh, PartitionSpec
        from jax.experimental.shard_map import shard_map
        from concourse.bass2jax import (
            _bass_exec_p,
            install_neuronx_cc_hook,
            partition_id_tensor,
        )

        install_neuronx_cc_hook()
        self.jax = jax
        self.n_cores = n_cores
        partition_name = (
            nc.partition_id_tensor.name if nc.partition_id_tensor else None
        )
        in_names, out_names, out_avals, zero_outs = [], [], [], []
        for alloc in nc.m.functions[0].allocations:
            if not isinstance(alloc, mybir.MemoryLocationSet):
                continue
            name = alloc.memorylocations[0].name
            if alloc.kind == "ExternalInput":
                if name != partition_name:
                    in_names.append(name)
            elif alloc.kind == "ExternalOutput":
                out_names.append(name)
                shape = tuple(alloc.tensor_shape)
                dtype = mybir.dt.np(alloc.dtype)
                out_avals.append(jax.core.ShapedArray(shape, dtype))
                zero_outs.append(np.zeros(shape, dtype))
        self.in_names = in_names
        self.out_names = out_names
        self.out_avals = out_avals
        self.zero_outs = zero_outs
        n_params = len(in_names)
        n_outs = len(out_avals)
        all_in_names = in_names + out_names
        if partition_name is not None:
            all_in_names.append(partition_name)

        def _body(*args):
            operands = list(args)
            if partition_name is not None:
                operands.append(partition_id_tensor())
            outs = _bass_exec_p.bind(
                *operands,
                out_avals=tuple(out_avals),
                in_names=tuple(all_in_names),
                out_names=tuple(out_names),
                lowering_input_output_aliases=(),
                sim_require_finite=True,
                sim_require_nnan=True,
                nc=nc,
            )
            return tuple(outs)

        devices = jax.devices()[:n_cores]
        assert len(devices) == n_cores, f"need {n_cores} cores"
        self.mesh = Mesh(np.asarray(devices), ("core",))
        in_specs = (PartitionSpec("core"),) * (n_params + n_outs)
        out_specs = (PartitionSpec("core"),) * len(out_names)
        self.fn = jax.jit(
            shard_map(
                _body, mesh=self.mesh, in_specs=in_specs, out_specs=out_specs,
                check_rep=False,
            ),
            keep_unused=True,
        )

    def prepare(self, in_maps):
        import jax
        from jax.sharding import PartitionSpec

        n = self.n_cores
        concat = [
            np.concatenate([np.asarray(in_maps[c][name]) for c in range(n)], axis=0)
            for name in self.in_names
        ] + [
            np.zeros((n * z.shape[0], *z.shape[1:]), z.dtype) for z in self.zero_outs
        ]
        sharding = jax.sharding.NamedSharding(self.mesh, PartitionSpec("core"))
        return [jax.device_put(a, sharding) for a in concat]

    def run(self, args):
        out = self.fn(*args)
        self.jax.block_until_ready(out)
        return [
            {
                name: np.asarray(out[i]).reshape(
                    self.n_cores, *self.out_avals[i].shape
                )[c]
                for i, name in enumerate(self.out_names)
            }
            for c in range(self.n_cores)
        ]


FP8 = True


def _get_runner(reps: int = 1, fp8: bool | None = None):
    if fp8 is None:
        fp8 = FP8
    key = ("runner", reps, fp8)
    if key not in _CACHE:
        nc = _build_nc(reps, fp8)
        _CACHE[key] = _SpmdRunner(nc, N_CORES)
    return _CACHE[key]


def _prep_host(input_ids, attack_mask, attack, emb_weight, fp8: bool | None = None):
    """Gather/scatter + per-core layout prep (host-side data movement)."""
    if fp8 is None:
        fp8 = FP8
    bf16 = ml_dtypes.bfloat16
    sweep_np = ml_dtypes.float8_e4m3 if fp8 else bf16
    W = np.ascontiguousarray(np.asarray(emb_weight, dtype=np.float32))
    ids = np.asarray(input_ids).astype(np.int64)
    mask = np.asarray(attack_mask).astype(bool)
    A = np.asarray(attack, dtype=np.float32)

    # reference semantics: embed, then masked scatter of tiled attack rows
    X = W[ids.reshape(-1)].reshape(B, S, D).copy()
    rank = (np.cumsum(mask, axis=-1) - 1) % N_ATK
    X[mask] = A[rank[mask]]
    embeds = X.copy()  # fp32 exact output
    Xq = X.reshape(BSQ, D)

    # query side, sweep dtype, blocked [QT, P(part=d), KT, P(free=q)]
    X_sw = Xq.astype(sweep_np)
    xt = np.ascontiguousarray(
        X_sw.reshape(QT, P, KT, P).transpose(0, 3, 2, 1)
    )  # [qt, p, k, j] = X[qt*128+j, k*128+p]

    # attack rows hi/lo, padded to 32, blocked [P, KT, NA]
    A_hi = A.astype(bf16)
    A_lo = (A - A_hi.astype(np.float32)).astype(bf16)

    def att_block(Ax):
        Ap = np.zeros((NA, D), dtype=bf16)
        Ap[:N_ATK] = Ax
        return np.ascontiguousarray(Ap.reshape(NA, KT, P).transpose(2, 1, 0))

    xat = att_block(A_hi)
    xalo = att_block(A_lo)

    in_maps = []
    for c in range(N_CORES):
        Wc = W[c * VSH : (c + 1) * VSH]
        Whi = Wc.astype(bf16)
        Wlo = (Wc - Whi.astype(np.float32)).astype(bf16)

        Whi_p = np.zeros((VPAD, D), dtype=bf16)
        Whi_p[:VSH] = Whi
        Wlo_p = np.zeros((VPAD, D), dtype=bf16)
        Wlo_p[:VSH] = Wlo
        Wsw_p = np.zeros((VPAD, D), dtype=sweep_np)
        Wsw_p[:VSH] = Wc.astype(sweep_np)
        # wt: [P(part=d), NV, KT, 512]; whi/wlo: [NV, KT, P, 512]
        wt = np.ascontiguousarray(
            Wsw_p.reshape(NV, 512, KT, P).transpose(3, 0, 2, 1)
        )
        whi = np.ascontiguousarray(
            Whi_p.reshape(NV, 512, KT, P).transpose(0, 2, 3, 1)
        )
        wlo = np.ascontiguousarray(
            Wlo_p.reshape(NV, 512, KT, P).transpose(0, 2, 3, 1)
        )
        w2n = np.full((VPAD,), NEG_BIG, dtype=np.float32)
        w2n[:VSH] = -0.5 * np.einsum(
            "vd,vd->v", Wc.astype(np.float64), Wc.astype(np.float64)
        ).astype(np.float32)
        w2rep = np.ascontiguousarray(np.broadcast_to(w2n, (P, VPAD)))

        in_maps.append(
            {
                "xt": xt,
                "wt": wt,
                "whi": whi,
                "wlo": wlo,
                "xat": xat,
                "xalo": xalo,
                "w2rep": w2rep,
            }
        )
    return embeds, rank, mask, in_maps


def _combine(results, rank, mask, ids_dtype):
    """Host-side all-reduce argmin over the 8 local argmins + attack override."""
    vals = np.stack([results[c]["mval"].reshape(BSQ) for c in range(N_CORES)])
    idxs = np.stack([results[c]["midx"].reshape(BSQ) for c in range(N_CORES)])
    best_core = np.argmax(vals, axis=0)  # ties -> lowest core == lowest vocab id
    adv = best_core * VSH + idxs[best_core, np.arange(BSQ)]

    avals = np.stack([results[c]["aval"][:N_ATK] for c in range(N_CORES)])
    aidxs = np.stack([results[c]["aidx"][:N_ATK] for c in range(N_CORES)])
    abest_core = np.argmax(avals, axis=0)
    att_ids = abest_core * VSH + aidxs[abest_core, np.arange(N_ATK)]

    adv = adv.reshape(B, S)
    adv[mask] = att_ids[rank[mask]]
    return adv.astype(ids_dtype)


def kernel(input_ids, attack_mask, attack, emb_weight):
    embeds, rank, mask, in_maps = _prep_host(
        input_ids, attack_mask, attack, emb_weight
    )
    runner = _get_runner(reps=1)
    args = runner.prepare(in_maps)
    results = runner.run(args)
    adv = _combine(results, rank, mask, np.asarray(input_ids).dtype)
    return embeds, adv
